# revision 8
# baseline (speedup 1.0000x reference)
"""GNN message-passing kernel v3 for Trainium2, 8 NeuronCores.

- fp16 bank tables (x + SHIFT), 4 banks of <=32768 rows (int16 gather idx),
  per-core edge coloring onto banks.
- dma_gather transpose=True (<=512 idx/instr, 4 SWDGE queues) produces
  [feat, slot] fp16 tiles directly -- no PE transposes, no PSUM staging.
- Adaptive degree-quantized windows: within a window every node has the
  same padded degree T* (pad budget bounds waste), so each (window, bank)
  needs exactly one uniform-width segmented reduce per sub-gather.
- bank0 reduces write fp16 accumulators directly; banks 1-3 reduce into
  tmp tiles and combine (max/add).
- Projection per 128 nodes: fp16 matmuls into PSUM, transpose, invd scale,
  bias, log_softmax without max-shift (logits are small), DMA out.
"""

import os
import sys

os.environ.setdefault("NEURON_RT_RESET_CORES", "1")
if "/opt/trn_rl_repo" not in sys.path:
    sys.path.insert(0, "/opt/trn_rl_repo")

import numpy as np

import concourse.mybir as mybir
from concourse import bacc, bass, tile
from concourse.masks import make_identity

N_NODES = 100000
D = 128
NCLS = 40
NCORES = 8
NPC = 12500
NPAD = 12544
NPROJ = NPAD // 128  # 98
NBANK = 4
TBL_ROWS = 32768
GMAX = 512        # dma_gather transpose-mode idx limit
WCAP = 2048       # max bank0 slots per window
PADBUDGET = 96    # quantization pad slots per window
SHIFT = 12.0

last_exec_time_ns = None


def _plan_template(dst):
    core = dst // NPC
    degs = np.zeros((NCORES, NPAD), np.int64)
    orders = np.zeros((NCORES, NPAD), np.int64)
    for c in range(NCORES):
        degs[c, :NPC] = np.bincount(dst[core == c] - c * NPC, minlength=NPC)
        orders[c] = np.argsort(degs[c], kind="stable")
    sdeg = np.take_along_axis(degs, orders, axis=1)
    T = sdeg.max(axis=0)

    # adaptive windows: uniform quantized degree T* per window
    windows = []  # (pa, pb, Tq)
    pa = 0
    while pa < NPAD:
        pb = pa + 1
        while pb < NPAD:
            Tq = int(T[pb])  # T sorted ascending -> max is at pb
            n = pb + 1 - pa
            pad = Tq * n - int(T[pa:pb + 1].sum())
            b0 = (Tq + 3) // 4 * n
            if pad > PADBUDGET or b0 > WCAP:
                break
            pb += 1
        Tq = int(T[pb - 1])
        windows.append((pa, pb, max(Tq, 1)))
        pa = pb

    # per (window, bank): d_b, sub-gathers
    chunks = []  # per window: list of per-bank dicts
    col0 = 0
    for (pa, pb, Tq) in windows:
        n = pb - pa
        wlist = []
        for b in range(NBANK):
            d = (Tq + 3 - b) // 4
            if d == 0:
                wlist.append(dict(b=b, d=0, subs=[]))
                continue
            npsub = max(1, GMAX // d)
            subs = []
            na = pa
            while na < pb:
                nb_ = min(na + npsub, pb)
                L = (nb_ - na) * d
                Lpad = -(-L // 128) * 128
                subs.append(dict(na=na, nb=nb_, L=L, Lpad=Lpad, col0=col0))
                col0 += Lpad // 16
                na = nb_
            wlist.append(dict(b=b, d=d, subs=subs))
        chunks.append(wlist)
    ncols = col0

    proj_after = []
    pc = 0
    for wi, (pa, pb, Tq) in enumerate(windows):
        lst = []
        while pc < NPROJ and (pc + 1) * 128 <= pb:
            lst.append(pc)
            pc += 1
        proj_after.append(lst)
    while pc < NPROJ:
        proj_after[-1].append(pc)
        pc += 1

    # per-position bank capacity (for coloring): node at position p in
    # window w gets cap (Tq(w)+3-b)//4 for bank b
    Tb = np.zeros((NBANK, NPAD), np.int64)
    for (pa, pb, Tq) in windows:
        for b in range(NBANK):
            Tb[b, pa:pb] = (Tq + 3 - b) // 4
    return degs, orders, sdeg, T, Tb, windows, chunks, ncols, proj_after


def _color_core(src_c, dstloc, pos, Tb):
    ne = len(src_c)
    p_e = pos[dstloc]
    rem = Tb.copy()
    bank = np.full(ne, -1, np.int8)

    order = np.argsort(src_c, kind="stable")
    s_sorted = src_c[order]
    uniq, start, cnt = np.unique(s_sorted, return_index=True,
                                 return_counts=True)
    src_rank = np.argsort(-cnt, kind="stable")
    rows_used = np.zeros(NBANK, np.int64)
    multi = src_rank[cnt[src_rank] >= 2]
    singles = src_rank[cnt[src_rank] == 1]

    for si in multi:
        a, k = start[si], cnt[si]
        eidx = order[a:a + k]
        ps = p_e[eidx]
        up, ucnt = np.unique(ps, return_counts=True)
        fits = (rem[:, up] >= ucnt).all(axis=1)
        if fits.any():
            fb = np.flatnonzero(fits)
            b = fb[np.argmin(rows_used[fb])]
            bank[eidx] = b
            np.subtract.at(rem[b], up, ucnt)
            rows_used[b] += 1
        else:
            used = set()
            for e in eidx:
                pe = p_e[e]
                b = int(np.argmax(rem[:, pe]))
                bank[e] = b
                rem[b, pe] -= 1
                used.add(b)
            rows_used[list(used)] += 1

    se = order[start[singles]]
    for chunk in np.array_split(se, max(1, len(se) // 20000)):
        pe = p_e[chunk]
        b = np.argmax(rem[:, pe], axis=0)
        np.subtract.at(rem, (b, pe), 1)
        bank[chunk] = b
        bad = np.argwhere(rem < 0)
        for bb, pp in bad:
            over = -rem[bb, pp]
            cand = chunk[(pe == pp) & (bank[chunk] == bb)]
            for e in cand[:over]:
                nb = int(np.argmax(rem[:, pp]))
                bank[e] = nb
                rem[nb, pp] -= 1
                rem[bb, pp] += 1
    assert (bank >= 0).all()
    assert (rem >= 0).all()
    return bank


def _core_data(x, src_c, dstloc, pos, Tb, windows, chunks, ncols):
    bank = _color_core(src_c, dstloc, pos, Tb)
    p_e = pos[dstloc]

    tables = []
    rows = []
    for b in range(NBANK):
        used = np.unique(src_c[bank == b])
        assert len(used) + 1 <= TBL_ROWS, f"bank {b} overflow: {len(used)}"
        tbl = np.zeros((TBL_ROWS, D), np.float16)
        tbl[1:1 + len(used)] = (x[used] + SHIFT).astype(np.float16)
        tables.append(tbl)
        rows.append(used)

    idx_flat = np.zeros(ncols * 16, np.int16)
    for wi, wlist in enumerate(chunks):
        for ch in wlist:
            b, d = ch["b"], ch["d"]
            if d == 0:
                continue
            for sub in ch["subs"]:
                na, nb_, L, Lpad, col0 = (sub["na"], sub["nb"], sub["L"],
                                          sub["Lpad"], sub["col0"])
                m = (bank == b) & (p_e >= na) & (p_e < nb_)
                blk = np.zeros(Lpad, np.int16)
                if m.any():
                    eidx = np.flatnonzero(m)
                    pe = p_e[eidx]
                    o = np.argsort(pe, kind="stable")
                    eidx = eidx[o]
                    pe = pe[o]
                    grp = np.concatenate([[0],
                                          np.flatnonzero(np.diff(pe)) + 1])
                    sizes = np.diff(np.concatenate([grp, [len(pe)]]))
                    rank = np.arange(len(pe)) - np.repeat(grp, sizes)
                    slots = (pe - na) * d + rank
                    blk[slots] = (np.searchsorted(rows[b], src_c[eidx])
                                  + 1).astype(np.int16)
                idx_flat[col0 * 16: col0 * 16 + Lpad] = blk
    wrapped = idx_flat.reshape(ncols, 16).T
    idx_arr = np.tile(wrapped, (8, 1)).astype(np.int16)
    return tables, idx_arr


def _build_program(windows, chunks, proj_after, ncols):
    nc = bacc.Bacc(num_swdge_queues=4)
    f32 = mybir.dt.float32
    f16 = mybir.dt.float16
    i16 = mybir.dt.int16

    tbl_in = [nc.declare_dram_parameter(f"tbl{b}", [TBL_ROWS, D], f16,
                                        isOutput=False) for b in range(NBANK)]
    idx_in = nc.declare_dram_parameter("idx", [128, ncols], i16,
                                       isOutput=False)
    xT_in = nc.declare_dram_parameter("xT", [D, NPAD], f16, isOutput=False)
    invd_in = nc.declare_dram_parameter("invd", [128, NPROJ], f32,
                                        isOutput=False)
    fix_in = nc.declare_dram_parameter("fix", [128, NPROJ, NCLS], f32,
                                       isOutput=False)
    wlmaxT_in = nc.declare_dram_parameter("wlmaxT", [D, NCLS], f16,
                                          isOutput=False)
    wlmeanT_in = nc.declare_dram_parameter("wlmeanT", [D, NCLS], f16,
                                           isOutput=False)
    wrcT_in = nc.declare_dram_parameter("wrcT", [D, NCLS], f16,
                                        isOutput=False)
    o_out = nc.declare_dram_parameter("out", [NPAD, NCLS], f32, isOutput=True)

    with tile.TileContext(nc) as tc, \
         nc.allow_low_precision(reason="fp16 aggregation validated offline"):
        with tc.tile_pool(name="persist", bufs=1) as pers:
            idx_t = pers.tile([128, ncols], i16)
            invd_t = pers.tile([128, NPROJ], f32)
            fix_t = pers.tile([128, NPROJ, NCLS], f32)
            wlmaxT_t = pers.tile([D, NCLS], f16)
            wlmeanT_t = pers.tile([D, NCLS], f16)
            wrcT_t = pers.tile([D, NCLS], f16)
            ident_t = pers.tile([128, 128], f32)
            acc_max = pers.tile([128, NPAD], f16)
            acc_sum = pers.tile([128, NPAD], f16)

            nc.sync.dma_start(out=idx_t[:, :], in_=idx_in[:, :])
            nc.sync.dma_start(out=invd_t[:, :], in_=invd_in[:, :])
            nc.sync.dma_start(out=fix_t[:, :, :], in_=fix_in[:, :, :])
            nc.sync.dma_start(out=wlmaxT_t[:, :], in_=wlmaxT_in[:, :])
            nc.sync.dma_start(out=wlmeanT_t[:, :], in_=wlmeanT_in[:, :])
            nc.sync.dma_start(out=wrcT_t[:, :], in_=wrcT_in[:, :])
            make_identity(nc, ident_t)
            nc.vector.memset(acc_max[:, :], 0.0)
            nc.vector.memset(acc_sum[:, :], 0.0)

            gather_seq = [0]

            with tc.tile_pool(name="gath", bufs=12) as gpool, \
                 tc.tile_pool(name="tmp", bufs=6) as tpool, \
                 tc.tile_pool(name="proj", bufs=2) as proj, \
                 tc.tile_pool(name="ppsum", bufs=2, space="PSUM") as prps:

                def emit_proj(pc):
                    c0 = pc * 128
                    xT_t = proj.tile([D, 128], f16, name="xTc")
                    nc.sync.dma_start(out=xT_t[:, :], in_=xT_in[:, c0:c0 + 128])

                    ps = prps.tile([128, 336], f32, name="ps")
                    nc.tensor.matmul(ps[:NCLS, 0:128], wlmeanT_t[:, :],
                                     acc_sum[:, c0:c0 + 128],
                                     start=True, stop=True)
                    nc.tensor.matmul(ps[:NCLS, 128:256], wlmaxT_t[:, :],
                                     acc_max[:, c0:c0 + 128],
                                     start=True, stop=False)
                    nc.tensor.matmul(ps[:NCLS, 128:256], wrcT_t[:, :],
                                     xT_t[:, :], start=False, stop=True)

                    sA = proj.tile([NCLS, 128], f32, name="sA")
                    sB = proj.tile([NCLS, 128], f32, name="sB")
                    nc.scalar.copy(sA[:, :], ps[:NCLS, 0:128])
                    nc.scalar.copy(sB[:, :], ps[:NCLS, 128:256])
                    nc.tensor.transpose(ps[:, 256:296], sA[:, :],
                                        ident_t[:NCLS, :NCLS])
                    nc.tensor.transpose(ps[:, 296:336], sB[:, :],
                                        ident_t[:NCLS, :NCLS])

                    z = proj.tile([128, NCLS], f32, name="z")
                    nc.vector.tensor_scalar(
                        out=z[:, :], in0=ps[:, 256:296],
                        scalar1=invd_t[:, pc:pc + 1], scalar2=None,
                        op0=mybir.AluOpType.mult,
                    )
                    nc.vector.tensor_tensor(z[:, :], z[:, :], ps[:, 296:336],
                                            mybir.AluOpType.add)
                    nc.vector.tensor_tensor(z[:, :], z[:, :], fix_t[:, pc, :],
                                            mybir.AluOpType.add)

                    # log_softmax without max-shift: |z| is small
                    e = proj.tile([128, NCLS], f32, name="e")
                    se = proj.tile([128, 1], f32, name="se")
                    nc.scalar.activation(
                        e[:, :], z[:, :], mybir.ActivationFunctionType.Exp,
                        scale=1.0, accum_out=se[:, :1],
                    )
                    ls = proj.tile([128, 1], f32, name="ls")
                    nc.scalar.activation(ls[:, :], se[:, :],
                                         mybir.ActivationFunctionType.Ln)
                    ot = proj.tile([128, NCLS], f32, name="ot")
                    nc.vector.tensor_scalar(
                        out=ot[:, :], in0=z[:, :], scalar1=ls[:, :1],
                        scalar2=None, op0=mybir.AluOpType.subtract,
                    )
                    nc.sync.dma_start(out=o_out[c0:c0 + 128, :], in_=ot[:, :])

                for wi, (pa, pb, Tq) in enumerate(windows):
                    n = pb - pa
                    for ch in chunks[wi]:
                        b, d = ch["b"], ch["d"]
                        if d == 0:
                            continue
                        if b == 0:
                            for sub in ch["subs"]:
                                na, nb_ = sub["na"], sub["nb"]
                                L, Lpad, col0 = sub["L"], sub["Lpad"], sub["col0"]
                                g = gpool.tile([128, 1, Lpad], f16, name="g")
                                nc.gpsimd.dma_gather(
                                    g[:, :, :], tbl_in[b][:, :],
                                    idx_t[:, col0:col0 + Lpad // 16],
                                    Lpad, Lpad, D, transpose=True,
                                    queue_num=gather_seq[0] % 4,
                                )
                                gather_seq[0] += 1
                                seg = g[:, 0, 0:L].rearrange(
                                    "p (nb d) -> p nb d", d=d)
                                nc.vector.tensor_reduce(
                                    out=acc_max[:, na:nb_], in_=seg,
                                    axis=mybir.AxisListType.X,
                                    op=mybir.AluOpType.max)
                                nc.vector.tensor_reduce(
                                    out=acc_sum[:, na:nb_], in_=seg,
                                    axis=mybir.AxisListType.X,
                                    op=mybir.AluOpType.add)
                        else:
                            tm = tpool.tile([128, n], f16, name="tm")
                            ts = tpool.tile([128, n], f16, name="ts")
                            for sub in ch["subs"]:
                                na, nb_ = sub["na"], sub["nb"]
                                L, Lpad, col0 = sub["L"], sub["Lpad"], sub["col0"]
                                g = gpool.tile([128, 1, Lpad], f16, name="g")
                                nc.gpsimd.dma_gather(
                                    g[:, :, :], tbl_in[b][:, :],
                                    idx_t[:, col0:col0 + Lpad // 16],
                                    Lpad, Lpad, D, transpose=True,
                                    queue_num=gather_seq[0] % 4,
                                )
                                gather_seq[0] += 1
                                seg = g[:, 0, 0:L].rearrange(
                                    "p (nb d) -> p nb d", d=d)
                                nc.vector.tensor_reduce(
                                    out=tm[:, na - pa:nb_ - pa], in_=seg,
                                    axis=mybir.AxisListType.X,
                                    op=mybir.AluOpType.max)
                                nc.vector.tensor_reduce(
                                    out=ts[:, na - pa:nb_ - pa], in_=seg,
                                    axis=mybir.AxisListType.X,
                                    op=mybir.AluOpType.add)
                            nc.vector.tensor_tensor(
                                acc_max[:, pa:pb], acc_max[:, pa:pb],
                                tm[:, :], mybir.AluOpType.max)
                            nc.vector.tensor_tensor(
                                acc_sum[:, pa:pb], acc_sum[:, pa:pb],
                                ts[:, :], mybir.AluOpType.add)
                    for pc in proj_after[wi]:
                        emit_proj(pc)
    return nc


def kernel(**inputs):
    global last_exec_time_ns
    x = np.asarray(inputs["x"], dtype=np.float32)
    ei = np.asarray(inputs["edge_index"]).astype(np.int64)
    Wl_max = np.asarray(inputs["Wl_max"], dtype=np.float32)
    Wr_max = np.asarray(inputs["Wr_max"], dtype=np.float32)
    b_max = np.asarray(inputs["b_max"], dtype=np.float32)
    Wl_mean = np.asarray(inputs["Wl_mean"], dtype=np.float32)
    Wr_mean = np.asarray(inputs["Wr_mean"], dtype=np.float32)
    b_mean = np.asarray(inputs["b_mean"], dtype=np.float32)

    src, dst = ei[0], ei[1]
    (degs, orders, sdeg, T, Tb, windows, chunks, ncols,
     proj_after) = _plan_template(dst)

    # fp16 weights; bias correction uses the fp16-rounded weights
    wlmaxT16 = np.ascontiguousarray(Wl_max.T).astype(np.float16)
    wlmeanT16 = np.ascontiguousarray(Wl_mean.T).astype(np.float16)
    wrcT16 = np.ascontiguousarray((Wr_max + Wr_mean).T).astype(np.float16)
    rs = SHIFT * (wlmaxT16.astype(np.float32).sum(axis=0)
                  + wlmeanT16.astype(np.float32).sum(axis=0))
    bias_eff = b_max + b_mean - rs

    core = dst // NPC
    in_maps = []
    for c in range(NCORES):
        msk = core == c
        src_c = src[msk]
        dstloc = dst[msk] - c * NPC
        pos = np.empty(NPAD, np.int64)
        pos[orders[c]] = np.arange(NPAD)
        tables, idx_arr = _core_data(x, src_c, dstloc, pos, Tb, windows,
                                     chunks, ncols)

        ids = orders[c]
        real = ids < NPC
        xo = np.zeros((NPAD, D), np.float32)
        xo[real] = x[c * NPC + ids[real]]
        xT = np.ascontiguousarray(xo.T).astype(np.float16)

        invd = (1.0 / np.maximum(sdeg[c], 1)).astype(np.float32)
        invd_t = np.ascontiguousarray(invd.reshape(NPROJ, 128).T)

        fix = np.tile(bias_eff, (NPAD, 1)).astype(np.float32)
        fix[sdeg[c] == 0] += rs
        fix_t = np.ascontiguousarray(
            fix.reshape(NPROJ, 128, NCLS).transpose(1, 0, 2))

        im = {"idx": idx_arr, "xT": xT, "invd": invd_t, "fix": fix_t,
              "wlmaxT": wlmaxT16, "wlmeanT": wlmeanT16, "wrcT": wrcT16}
        for b in range(NBANK):
            im[f"tbl{b}"] = tables[b]
        in_maps.append(im)

    nc = _build_program(windows, chunks, proj_after, ncols)
    nc.compile()

    from concourse.bass_utils import run_bass_kernel_spmd
    res = run_bass_kernel_spmd(nc, in_maps, list(range(NCORES)))
    if os.environ.get("GNN_TRACE", "0") == "1":
        tr = run_bass_kernel_spmd(nc, in_maps[:1], [0], trace=True)
        last_exec_time_ns = tr.exec_time_ns

    out = np.zeros((N_NODES, NCLS), np.float32)
    for c in range(NCORES):
        o = np.asarray(res.results[c]["out"])
        ids = orders[c]
        real = ids < NPC
        out[c * NPC + ids[real]] = o[real]
    return out
